# revision 20
# baseline (speedup 1.0000x reference)
"""Sparse-attention (multiplicative mask) kernel for 8 TRN2 NeuronCores.

Reference computation (B=2, N=2048, C=1024, H=16, D=64):
    qkv = x @ w_qkv.T -> q,k,v [B,H,N,D]
    attn = softmax((q @ k.T) * D**-0.5 * mask)   # mask multiplies scores
    out  = (attn @ v) reshaped -> @ w_proj.T + b_proj

Sharding: core c handles batch b = c//4 and heads 4*(c%4) .. 4*(c%4)+3.
Each core computes a partial output projection (its 256 head-dims x full
w_proj columns); host sums the 4 partials per batch and adds b_proj.

Per-core device pipeline (bf16 matmul operands, fp32 softmax core):
  xT[c,n] resident; Q^T/K^T computed pair-packed [128,n] directly
  (M=128 matmuls spanning both heads' weight columns; scale folded
  into Q); V natural + ones column [128,nk,65] bf16.
  Attention per (pair, q-half): scores S^T[128,1024] on PE (row-tiled
  per head) -> DVE multiply by mask^T (bf16) -> ACT exp -> P^T bf16;
  AV accumulates out^T[65,1024] on PE two k-chunks behind the exp
  chain (deep software pipeline; the ones column of V gives the
  softmax denominator). At each q-half boundary av is quickly evicted
  to SBUF to free PSUM; the reciprocal + broadcast + normalize runs
  deferred, off the critical path. AO is pair-packed via SBUF->SBUF
  DMA for a K=128 output projection.
"""

import os

os.environ.setdefault("MYCRO_LOCAL_CACHE", "1")

import numpy as np
import ml_dtypes

import concourse.bass as bass
from concourse import bacc
import concourse.mybir as mybir
import concourse.tile as tile
from concourse.bass_utils import run_bass_kernel_spmd

B = 2
N = 2048
C = 1024
H = 16
D = 64
HPC = 4  # heads per core
NCORES = 8
SCALE = float(D) ** -0.5

F32 = mybir.dt.float32
BF16 = mybir.dt.bfloat16
EXP = mybir.ActivationFunctionType.Exp


def build_nc(n=N):
    """Build the per-core Bass program (SPMD; per-core inputs differ)."""
    nc = bacc.Bacc()

    NCC = C // 128        # contraction chunks for qkv proj (8)
    NK = n // 128         # key chunks (16)
    QH = n // 2           # half of the query range (1024)
    NQC = QH // 512       # 512-wide q chunks per half (2)

    xT = nc.dram_tensor("xT", [C, n], BF16, kind="ExternalInput")
    wqT = nc.dram_tensor("wqT", [C, HPC * D], BF16, kind="ExternalInput")
    wkT = nc.dram_tensor("wkT", [C, HPC * D], BF16, kind="ExternalInput")
    wvT = nc.dram_tensor("wvT", [C, HPC * D], BF16, kind="ExternalInput")
    wpT = nc.dram_tensor("wpT", [HPC * D, C], BF16, kind="ExternalInput")
    maskT = nc.dram_tensor("maskT", [n, n], BF16, kind="ExternalInput")
    y = nc.dram_tensor("y", [n, C], F32, kind="ExternalOutput")

    with tile.TileContext(nc) as tc:
        with (
            tc.tile_pool(name="weights", bufs=NCC) as wpool,
            tc.tile_pool(name="xt", bufs=NCC) as xpool,
            tc.tile_pool(name="qk", bufs=2) as qkpool,
            tc.tile_pool(name="vaug", bufs=HPC) as vpool,
            tc.tile_pool(name="ao", bufs=2) as aopool,
            tc.tile_pool(name="mask", bufs=3) as mpool,
            tc.tile_pool(name="work", bufs=3) as workpool,
            tc.tile_pool(name="small", bufs=4) as smallpool,
            tc.tile_pool(name="dscratch", bufs=4, space="DRAM") as dpool,
            tc.tile_pool(name="yout", bufs=3) as ypool,
        ):
            # ---- load weights + xT ----
            wq_sb = []
            wk_sb = []
            wv_sb = []
            x_sb = []
            for c in range(NCC):
                wq = wpool.tile([128, HPC * D], BF16, tag="wq")
                nc.sync.dma_start(out=wq, in_=wqT[c * 128:(c + 1) * 128, :])
                wq_sb.append(wq)
                wk = wpool.tile([128, HPC * D], BF16, tag="wk")
                nc.sync.dma_start(out=wk, in_=wkT[c * 128:(c + 1) * 128, :])
                wk_sb.append(wk)
                wv = wpool.tile([128, HPC * D], BF16, tag="wv")
                nc.sync.dma_start(out=wv, in_=wvT[c * 128:(c + 1) * 128, :])
                wv_sb.append(wv)
                xt = xpool.tile([128, n], BF16, tag="xt")
                nc.sync.dma_start(out=xt, in_=xT[c * 128:(c + 1) * 128, :])
                x_sb.append(xt)
            wp_sb = []
            for pair in range(HPC // 2):
                wp = wpool.tile([128, C], BF16, tag="wp", bufs=2)
                nc.sync.dma_start(
                    out=wp, in_=wpT[pair * 128:(pair + 1) * 128, :])
                wp_sb.append(wp)

            # ---- V for all heads (+ ones column), bf16 ----
            vaug = []
            with tc.tile_pool(name="v_psum", bufs=2, space="PSUM") as vps:
                for h in range(HPC):
                    va = vpool.tile([128, NK, D + 1], BF16, tag="vaug")
                    nc.vector.memset(va[:, :, D:D + 1], 1.0)
                    vaug.append(va)
                for quarter in range(NK // 4):
                    # each [128, 256] f32 output padded to its own 2KB PSUM
                    # bank: matmul start clears the whole bank region
                    vp = vps.tile([128, 4, 512], F32, tag="vps")
                    for kk in range(4):
                        k = quarter * 4 + kk
                        for c in range(NCC):
                            nc.tensor.matmul(
                                vp[:, kk, :HPC * D],
                                lhsT=x_sb[c][:, k * 128:(k + 1) * 128],
                                rhs=wv_sb[c],
                                start=(c == 0),
                                stop=(c == NCC - 1),
                            )
                    for h in range(HPC):
                        nc.vector.tensor_copy(
                            vaug[h][:, quarter * 4:(quarter + 1) * 4, :D],
                            vp[:, :, h * D:(h + 1) * D],
                        )

            # ---- Q^T / K^T pair-packed [128, n]: one M=128 matmul spans
            # both heads' weight columns, so head 2p lands on partitions
            # 0-63 and head 2p+1 on 64-127 directly ----
            qt2 = {}
            kt2 = {}
            with tc.tile_pool(name="qk_psum", bufs=4, space="PSUM") as qkps:
                for pair in range(HPC // 2):
                    hs = slice(pair * 128, (pair + 1) * 128)
                    for which, wsb, dstmap, scale in (
                        ("q", wq_sb, qt2, SCALE),
                        ("k", wk_sb, kt2, 1.0),
                    ):
                        dst = qkpool.tile([128, n], BF16, tag=f"{which}t",
                                          name=f"{which}t")
                        dstmap[pair] = dst
                        for qc in range(n // 512):
                            ps = qkps.tile([128, 512], F32, tag="qkps",
                                           name="qkps")
                            for c in range(NCC):
                                nc.tensor.matmul(
                                    ps,
                                    lhsT=wsb[c][:, hs],
                                    rhs=x_sb[c][:, qc * 512:(qc + 1) * 512],
                                    start=(c == 0),
                                    stop=(c == NCC - 1),
                                )
                            nc.vector.tensor_scalar_mul(
                                dst[:, qc * 512:(qc + 1) * 512], ps, scale)

            ao2 = {}
            for pair in range(HPC // 2):
                ao2[pair] = aopool.tile([128, n], BF16, tag="ao", name="ao2")

            # ---- attention ----
            deferred = []

            def emit_normalize(item):
                pair, qh, h, av_sb = item
                # reciprocal of the denominator row, reshaped to
                # [128, QH//128] via DRAM so all DVE lanes work
                rrd = dpool.tile([1, QH], F32, tag="rrd", name="rrd")
                nc.sync.dma_start(out=rrd, in_=av_sb[D:D + 1, :])
                grid = [[QH // 128, 128], [1, QH // 128]]
                rcol = smallpool.tile([128, QH // 128], F32,
                                      tag="rcol", name="rcol")
                nc.sync.dma_start(
                    out=rcol,
                    in_=bass.AP(tensor=rrd.tensor, offset=rrd.offset,
                                ap=[list(x) for x in grid]),
                )
                rco = smallpool.tile([128, QH // 128], F32,
                                     tag="rco", name="rco")
                nc.vector.reciprocal(rco, rcol)
                rd2 = dpool.tile([1, QH], F32, tag="rd2", name="rd2")
                nc.sync.dma_start(
                    out=bass.AP(tensor=rd2.tensor, offset=rd2.offset,
                                ap=[list(x) for x in grid]),
                    in_=rco,
                )
                bc = smallpool.tile([64, QH], F32, tag="bc", name="bc")
                nc.sync.dma_start(
                    out=bc,
                    in_=bass.AP(tensor=rd2.tensor, offset=rd2.offset,
                                ap=[[0, 64]] + rd2.ap[1:]),
                )
                if h % 2 == 0:
                    nc.vector.tensor_mul(
                        ao2[pair][0:64, qh * QH:(qh + 1) * QH],
                        av_sb[:D, :], bc)
                else:
                    tmp = smallpool.tile([64, QH], BF16, tag="aotmp",
                                         name="aotmp", bufs=2)
                    nc.vector.tensor_mul(tmp, av_sb[:D, :], bc)
                    # compute engines can't cross partitions; DMA packs the
                    # odd head onto partitions 64-127
                    nc.sync.dma_start(
                        out=ao2[pair][64:128, qh * QH:(qh + 1) * QH],
                        in_=tmp)

            with (
                tc.tile_pool(name="s_psum", bufs=2, space="PSUM") as sps,
                tc.tile_pool(name="av_psum", bufs=2, space="PSUM") as avps,
            ):
                for qh in range(2):
                    # mask tiles for this q-half stay resident and are
                    # shared by both head pairs (one DMA pass per q-half)
                    mts = []
                    for k in range(NK):
                        mt = mpool.tile([128, QH], BF16, tag="mask",
                                        name="mt", bufs=NK + 2)
                        nc.sync.dma_start(
                            out=mt,
                            in_=maskT[k * 128:(k + 1) * 128,
                                      qh * QH:(qh + 1) * QH],
                        )
                        mts.append(mt)
                    for pair in range(HPC // 2):
                        heads = [2 * pair, 2 * pair + 1]
                        av = {}
                        for h in heads:
                            av[h] = avps.tile([D + 1, QH], F32, tag="av",
                                              name="av")
                        # AV matmuls run two k behind the scores->mask->exp
                        # chain so PE never stalls on ACT
                        pending = []

                        def flush_av(batch):
                            for (fh, fk, fp) in batch:
                                for qc in range(NQC):
                                    nc.tensor.matmul(
                                        av[fh][:, qc * 512:(qc + 1) * 512],
                                        lhsT=vaug[fh][:, fk, :],
                                        rhs=fp[:, qc * 512:(qc + 1) * 512],
                                        start=(fk == 0),
                                        stop=(fk == NK - 1),
                                    )

                        for k in range(NK):
                            s = {}
                            for i, h in enumerate(heads):
                                s[h] = sps.tile([128, QH], F32, tag="s",
                                                name="s")
                            for i, h in enumerate(heads):
                                for qc in range(NQC):
                                    nc.tensor.matmul(
                                        s[h][:, qc * 512:(qc + 1) * 512],
                                        lhsT=kt2[pair][i * 64:(i + 1) * 64,
                                                       k * 128:(k + 1) * 128],
                                        rhs=qt2[pair][i * 64:(i + 1) * 64,
                                                      qh * QH + qc * 512:
                                                      qh * QH + (qc + 1) * 512],
                                        start=True,
                                        stop=True,
                                        tile_position=(64 * i, 0),
                                    )
                            for h in heads:
                                t = workpool.tile([128, QH], F32, tag="t",
                                                  name="t")
                                nc.vector.tensor_mul(t, s[h], mts[k])
                                p = workpool.tile([128, QH], BF16, tag="p",
                                                  name="p", bufs=7)
                                nc.scalar.activation(p, t, EXP)
                                pending.append((h, k, p))
                            if len(pending) >= 6:
                                flush_av(pending[:2])
                                pending = pending[2:]
                        flush_av(pending)

                        # quick PSUM release: copy av to SBUF, defer the
                        # reciprocal/normalize chain off the critical path
                        for h in heads:
                            av_sb = smallpool.tile([D + 1, QH], F32,
                                                   tag="av_sb", name="av_sb",
                                                   bufs=4)
                            nc.scalar.copy(av_sb, av[h])
                            deferred.append((pair, qh, h, av_sb))
                        while len(deferred) > 2:
                            emit_normalize(deferred.pop(0))
            for item in deferred:
                emit_normalize(item)

            # ---- output projection (partial over this core's 256 dims) ----
            with tc.tile_pool(name="proj_psum", bufs=2, space="PSUM") as pps:
                for nt in range(n // 128):
                    pp = pps.tile([128, C], F32, tag="pp")
                    for pair in range(HPC // 2):
                        for ch in range(2):
                            nc.tensor.matmul(
                                pp[:, ch * 512:(ch + 1) * 512],
                                lhsT=ao2[pair][:, nt * 128:(nt + 1) * 128],
                                rhs=wp_sb[pair][:, ch * 512:(ch + 1) * 512],
                                start=(pair == 0),
                                stop=(pair == HPC // 2 - 1),
                            )
                    ysb = ypool.tile([128, C], F32, tag="ysb")
                    nc.scalar.copy(ysb, pp)
                    nc.sync.dma_start(out=y[nt * 128:(nt + 1) * 128, :], in_=ysb)

    nc.compile()
    return nc


def make_in_maps(x, w_qkv, mask, w_proj, n=N):
    """Host-side sharding: slice + transpose per-core inputs."""
    maskT = np.ascontiguousarray(mask.T).astype(ml_dtypes.bfloat16)
    xTs = [np.ascontiguousarray(x[b].T) for b in range(x.shape[0])]
    in_maps = []
    for core in range(NCORES):
        b = core // 4
        g = core % 4
        rows = slice(g * HPC * D, (g + 1) * HPC * D)
        bf = ml_dtypes.bfloat16
        in_maps.append({
            "xT": xTs[b].astype(bf),
            "wqT": np.ascontiguousarray(w_qkv[rows, :].T).astype(bf),
            "wkT": np.ascontiguousarray(w_qkv[C:2 * C][rows, :].T).astype(bf),
            "wvT": np.ascontiguousarray(w_qkv[2 * C:3 * C][rows, :].T).astype(bf),
            "wpT": np.ascontiguousarray(w_proj[:, rows].T).astype(bf),
            "maskT": maskT,
        })
    return in_maps


_NC_CACHE = {}


def _get_nc():
    if "nc" not in _NC_CACHE:
        _NC_CACHE["nc"] = build_nc()
    return _NC_CACHE["nc"]


def run_on_cores(x, w_qkv, w_proj, b_proj, mask, trace=False, trace_cores=None):
    nc = _get_nc()
    in_maps = make_in_maps(x, w_qkv, mask, w_proj)
    res = run_bass_kernel_spmd(
        nc,
        in_maps,
        core_ids=list(range(NCORES)),
        trace=trace,
        trace_cores=trace_cores,
    )
    out = np.zeros((B, N, C), dtype=np.float32)
    for core in range(NCORES):
        out[core // 4] += res.results[core]["y"]
    out += np.asarray(b_proj, dtype=np.float32)
    return out, res


def kernel(x, w_qkv, w_proj, b_proj, mask):
    x = np.asarray(x, dtype=np.float32)
    w_qkv = np.asarray(w_qkv, dtype=np.float32)
    w_proj = np.asarray(w_proj, dtype=np.float32)
    b_proj = np.asarray(b_proj, dtype=np.float32)
    mask = np.asarray(mask, dtype=np.float32)
    out, _ = run_on_cores(x, w_qkv, w_proj, b_proj, mask)
    return out


# revision 21
# speedup vs baseline: 1.0595x; 1.0595x over previous
"""Sparse-attention (multiplicative mask) kernel for 8 TRN2 NeuronCores.

Reference computation (B=2, N=2048, C=1024, H=16, D=64):
    qkv = x @ w_qkv.T -> q,k,v [B,H,N,D]
    attn = softmax((q @ k.T) * D**-0.5 * mask)   # mask multiplies scores
    out  = (attn @ v) reshaped -> @ w_proj.T + b_proj

Sharding: core c handles batch b = c//4 and heads 4*(c%4) .. 4*(c%4)+3.
Each core computes a partial output projection (its 256 head-dims x full
w_proj columns); host sums the 4 partials per batch and adds b_proj.

Per-core device pipeline (bf16 matmul operands, fp32 softmax core):
  xT[c,n] resident; Q^T/K^T computed pair-packed [128,n] directly
  (M=128 matmuls spanning both heads' weight columns; scale folded
  into Q); V natural + ones column [128,nk,65] bf16.
  Attention per (pair, q-half): scores S^T[128,1024] on PE (row-tiled
  per head) -> DVE multiply by mask^T (bf16) -> ACT exp -> P^T bf16;
  AV accumulates out^T[65,1024] on PE two k-chunks behind the exp
  chain (deep software pipeline; the ones column of V gives the
  softmax denominator). At each q-half boundary av is quickly evicted
  to SBUF to free PSUM; the reciprocal + broadcast + normalize runs
  deferred, off the critical path. AO is pair-packed via SBUF->SBUF
  DMA for a K=128 output projection.
"""

import os

os.environ.setdefault("MYCRO_LOCAL_CACHE", "1")

import numpy as np
import ml_dtypes

import concourse.bass as bass
from concourse import bacc
import concourse.mybir as mybir
import concourse.tile as tile
from concourse.bass_utils import run_bass_kernel_spmd

B = 2
N = 2048
C = 1024
H = 16
D = 64
HPC = 4  # heads per core
NCORES = 8
SCALE = float(D) ** -0.5

F32 = mybir.dt.float32
BF16 = mybir.dt.bfloat16
EXP = mybir.ActivationFunctionType.Exp


def build_nc(n=N):
    """Build the per-core Bass program (SPMD; per-core inputs differ)."""
    nc = bacc.Bacc()

    NCC = C // 128        # contraction chunks for qkv proj (8)
    NK = n // 128         # key chunks (16)
    QH = n // 2           # half of the query range (1024)
    NQC = QH // 512       # 512-wide q chunks per half (2)

    xT = nc.dram_tensor("xT", [C, n], BF16, kind="ExternalInput")
    wqT = nc.dram_tensor("wqT", [C, HPC * D], BF16, kind="ExternalInput")
    wkT = nc.dram_tensor("wkT", [C, HPC * D], BF16, kind="ExternalInput")
    wvT = nc.dram_tensor("wvT", [C, HPC * D], BF16, kind="ExternalInput")
    wpT = nc.dram_tensor("wpT", [HPC * D, C], BF16, kind="ExternalInput")
    maskT = nc.dram_tensor("maskT", [n, n], BF16, kind="ExternalInput")
    y = nc.dram_tensor("y", [n, C], F32, kind="ExternalOutput")

    with tile.TileContext(nc) as tc:
        with (
            tc.tile_pool(name="weights", bufs=NCC) as wpool,
            tc.tile_pool(name="xt", bufs=NCC) as xpool,
            tc.tile_pool(name="qk", bufs=2) as qkpool,
            tc.tile_pool(name="vaug", bufs=HPC) as vpool,
            tc.tile_pool(name="ao", bufs=2) as aopool,
            tc.tile_pool(name="mask", bufs=3) as mpool,
            tc.tile_pool(name="work", bufs=3) as workpool,
            tc.tile_pool(name="small", bufs=4) as smallpool,
            tc.tile_pool(name="dscratch", bufs=4, space="DRAM") as dpool,
            tc.tile_pool(name="yout", bufs=3) as ypool,
        ):
            # ---- load weights + xT ----
            wq_sb = []
            wk_sb = []
            wv_sb = []
            x_sb = []
            for c in range(NCC):
                wq = wpool.tile([128, HPC * D], BF16, tag="wq")
                nc.sync.dma_start(out=wq, in_=wqT[c * 128:(c + 1) * 128, :])
                wq_sb.append(wq)
                wk = wpool.tile([128, HPC * D], BF16, tag="wk")
                nc.sync.dma_start(out=wk, in_=wkT[c * 128:(c + 1) * 128, :])
                wk_sb.append(wk)
                wv = wpool.tile([128, HPC * D], BF16, tag="wv")
                nc.sync.dma_start(out=wv, in_=wvT[c * 128:(c + 1) * 128, :])
                wv_sb.append(wv)
                xt = xpool.tile([128, n], BF16, tag="xt")
                nc.sync.dma_start(out=xt, in_=xT[c * 128:(c + 1) * 128, :])
                x_sb.append(xt)
            wp_sb = []
            for pair in range(HPC // 2):
                wp = wpool.tile([128, C], BF16, tag="wp", bufs=2)
                nc.sync.dma_start(
                    out=wp, in_=wpT[pair * 128:(pair + 1) * 128, :])
                wp_sb.append(wp)

            # ---- V for all heads (+ ones column), bf16 ----
            vaug = []
            with tc.tile_pool(name="v_psum", bufs=2, space="PSUM") as vps:
                for h in range(HPC):
                    va = vpool.tile([128, NK, D + 1], BF16, tag="vaug")
                    nc.vector.memset(va[:, :, D:D + 1], 1.0)
                    vaug.append(va)
                for quarter in range(NK // 4):
                    # each [128, 256] f32 output padded to its own 2KB PSUM
                    # bank: matmul start clears the whole bank region
                    vp = vps.tile([128, 4, 512], F32, tag="vps")
                    for kk in range(4):
                        k = quarter * 4 + kk
                        for c in range(NCC):
                            nc.tensor.matmul(
                                vp[:, kk, :HPC * D],
                                lhsT=x_sb[c][:, k * 128:(k + 1) * 128],
                                rhs=wv_sb[c],
                                start=(c == 0),
                                stop=(c == NCC - 1),
                            )
                    for h in range(HPC):
                        nc.vector.tensor_copy(
                            vaug[h][:, quarter * 4:(quarter + 1) * 4, :D],
                            vp[:, :, h * D:(h + 1) * D],
                        )

            # ---- Q^T / K^T pair-packed [128, n]: one M=128 matmul spans
            # both heads' weight columns, so head 2p lands on partitions
            # 0-63 and head 2p+1 on 64-127 directly ----
            qt2 = {}
            kt2 = {}
            with tc.tile_pool(name="qk_psum", bufs=4, space="PSUM") as qkps:
                for pair in range(HPC // 2):
                    hs = slice(pair * 128, (pair + 1) * 128)
                    for which, wsb, dstmap, scale in (
                        ("q", wq_sb, qt2, SCALE),
                        ("k", wk_sb, kt2, 1.0),
                    ):
                        dst = qkpool.tile([128, n], BF16, tag=f"{which}t",
                                          name=f"{which}t")
                        dstmap[pair] = dst
                        for qc in range(n // 512):
                            ps = qkps.tile([128, 512], F32, tag="qkps",
                                           name="qkps")
                            for c in range(NCC):
                                nc.tensor.matmul(
                                    ps,
                                    lhsT=wsb[c][:, hs],
                                    rhs=x_sb[c][:, qc * 512:(qc + 1) * 512],
                                    start=(c == 0),
                                    stop=(c == NCC - 1),
                                )
                            nc.vector.tensor_scalar_mul(
                                dst[:, qc * 512:(qc + 1) * 512], ps, scale)

            ao2 = {}
            for pair in range(HPC // 2):
                ao2[pair] = aopool.tile([128, n], BF16, tag="ao", name="ao2")

            # ---- attention ----
            deferred = []

            def emit_normalize(item):
                pair, qh, h, av_sb = item
                # reciprocal of the denominator row, reshaped to
                # [128, QH//128] via DRAM so all DVE lanes work
                rrd = dpool.tile([1, QH], F32, tag="rrd", name="rrd")
                nc.sync.dma_start(out=rrd, in_=av_sb[D:D + 1, :])
                grid = [[QH // 128, 128], [1, QH // 128]]
                rcol = smallpool.tile([128, QH // 128], F32,
                                      tag="rcol", name="rcol")
                nc.sync.dma_start(
                    out=rcol,
                    in_=bass.AP(tensor=rrd.tensor, offset=rrd.offset,
                                ap=[list(x) for x in grid]),
                )
                rco = smallpool.tile([128, QH // 128], F32,
                                     tag="rco", name="rco")
                nc.vector.reciprocal(rco, rcol)
                rd2 = dpool.tile([1, QH], F32, tag="rd2", name="rd2")
                nc.sync.dma_start(
                    out=bass.AP(tensor=rd2.tensor, offset=rd2.offset,
                                ap=[list(x) for x in grid]),
                    in_=rco,
                )
                bc = smallpool.tile([64, QH], F32, tag="bc", name="bc")
                nc.sync.dma_start(
                    out=bc,
                    in_=bass.AP(tensor=rd2.tensor, offset=rd2.offset,
                                ap=[[0, 64]] + rd2.ap[1:]),
                )
                if h % 2 == 0:
                    nc.vector.tensor_mul(
                        ao2[pair][0:64, qh * QH:(qh + 1) * QH],
                        av_sb[:D, :], bc)
                else:
                    tmp = smallpool.tile([64, QH], BF16, tag="aotmp",
                                         name="aotmp", bufs=2)
                    nc.vector.tensor_mul(tmp, av_sb[:D, :], bc)
                    # compute engines can't cross partitions; DMA packs the
                    # odd head onto partitions 64-127
                    nc.sync.dma_start(
                        out=ao2[pair][64:128, qh * QH:(qh + 1) * QH],
                        in_=tmp)

            with (
                tc.tile_pool(name="s_psum", bufs=2, space="PSUM") as sps,
                tc.tile_pool(name="av_psum", bufs=2, space="PSUM") as avps,
            ):
                for pair in range(HPC // 2):
                    heads = [2 * pair, 2 * pair + 1]
                    for qh in range(2):
                        av = {}
                        for h in heads:
                            av[h] = avps.tile([D + 1, QH], F32, tag="av",
                                              name="av")
                        # AV matmuls run two k behind the scores->mask->exp
                        # chain so PE never stalls on ACT
                        pending = []

                        def flush_av(batch):
                            for (fh, fk, fp) in batch:
                                for qc in range(NQC):
                                    nc.tensor.matmul(
                                        av[fh][:, qc * 512:(qc + 1) * 512],
                                        lhsT=vaug[fh][:, fk, :],
                                        rhs=fp[:, qc * 512:(qc + 1) * 512],
                                        start=(fk == 0),
                                        stop=(fk == NK - 1),
                                    )

                        for k in range(NK):
                            mt = mpool.tile([128, QH], BF16, tag="mask",
                                            name="mt")
                            nc.sync.dma_start(
                                out=mt,
                                in_=maskT[k * 128:(k + 1) * 128,
                                          qh * QH:(qh + 1) * QH],
                            )
                            s = {}
                            for i, h in enumerate(heads):
                                s[h] = sps.tile([128, QH], F32, tag="s",
                                                name="s")
                            for qc in range(NQC):
                                for i, h in enumerate(heads):
                                    nc.tensor.matmul(
                                        s[h][:, qc * 512:(qc + 1) * 512],
                                        lhsT=kt2[pair][i * 64:(i + 1) * 64,
                                                       k * 128:(k + 1) * 128],
                                        rhs=qt2[pair][i * 64:(i + 1) * 64,
                                                      qh * QH + qc * 512:
                                                      qh * QH + (qc + 1) * 512],
                                        start=True,
                                        stop=True,
                                        tile_position=(64 * i, 0),
                                    )
                            for h in heads:
                                t = workpool.tile([128, QH], F32, tag="t",
                                                  name="t")
                                nc.vector.tensor_mul(t, s[h], mt)
                                p = workpool.tile([128, QH], BF16, tag="p",
                                                  name="p", bufs=7)
                                nc.scalar.activation(p, t, EXP)
                                pending.append((h, k, p))
                            if len(pending) >= 6:
                                flush_av(pending[:2])
                                pending = pending[2:]
                        flush_av(pending)

                        # quick PSUM release: copy av to SBUF, defer the
                        # reciprocal/normalize chain off the critical path
                        for h in heads:
                            av_sb = smallpool.tile([D + 1, QH], F32,
                                                   tag="av_sb", name="av_sb",
                                                   bufs=4)
                            nc.scalar.copy(av_sb, av[h])
                            deferred.append((pair, qh, h, av_sb))
                        while len(deferred) > 2:
                            emit_normalize(deferred.pop(0))
            for item in deferred:
                emit_normalize(item)

            # ---- output projection (partial over this core's 256 dims) ----
            with tc.tile_pool(name="proj_psum", bufs=2, space="PSUM") as pps:
                for nt in range(n // 128):
                    pp = pps.tile([128, C], F32, tag="pp")
                    for pair in range(HPC // 2):
                        for ch in range(2):
                            nc.tensor.matmul(
                                pp[:, ch * 512:(ch + 1) * 512],
                                lhsT=ao2[pair][:, nt * 128:(nt + 1) * 128],
                                rhs=wp_sb[pair][:, ch * 512:(ch + 1) * 512],
                                start=(pair == 0),
                                stop=(pair == HPC // 2 - 1),
                            )
                    ysb = ypool.tile([128, C], F32, tag="ysb")
                    nc.scalar.copy(ysb, pp)
                    nc.sync.dma_start(out=y[nt * 128:(nt + 1) * 128, :], in_=ysb)

    nc.compile()
    return nc


def make_in_maps(x, w_qkv, mask, w_proj, n=N):
    """Host-side sharding: slice + transpose per-core inputs."""
    maskT = np.ascontiguousarray(mask.T).astype(ml_dtypes.bfloat16)
    xTs = [np.ascontiguousarray(x[b].T) for b in range(x.shape[0])]
    in_maps = []
    for core in range(NCORES):
        b = core // 4
        g = core % 4
        rows = slice(g * HPC * D, (g + 1) * HPC * D)
        bf = ml_dtypes.bfloat16
        in_maps.append({
            "xT": xTs[b].astype(bf),
            "wqT": np.ascontiguousarray(w_qkv[rows, :].T).astype(bf),
            "wkT": np.ascontiguousarray(w_qkv[C:2 * C][rows, :].T).astype(bf),
            "wvT": np.ascontiguousarray(w_qkv[2 * C:3 * C][rows, :].T).astype(bf),
            "wpT": np.ascontiguousarray(w_proj[:, rows].T).astype(bf),
            "maskT": maskT,
        })
    return in_maps


_NC_CACHE = {}


def _get_nc():
    if "nc" not in _NC_CACHE:
        _NC_CACHE["nc"] = build_nc()
    return _NC_CACHE["nc"]


def run_on_cores(x, w_qkv, w_proj, b_proj, mask, trace=False, trace_cores=None):
    nc = _get_nc()
    in_maps = make_in_maps(x, w_qkv, mask, w_proj)
    res = run_bass_kernel_spmd(
        nc,
        in_maps,
        core_ids=list(range(NCORES)),
        trace=trace,
        trace_cores=trace_cores,
    )
    out = np.zeros((B, N, C), dtype=np.float32)
    for core in range(NCORES):
        out[core // 4] += res.results[core]["y"]
    out += np.asarray(b_proj, dtype=np.float32)
    return out, res


def kernel(x, w_qkv, w_proj, b_proj, mask):
    x = np.asarray(x, dtype=np.float32)
    w_qkv = np.asarray(w_qkv, dtype=np.float32)
    w_proj = np.asarray(w_proj, dtype=np.float32)
    b_proj = np.asarray(b_proj, dtype=np.float32)
    mask = np.asarray(mask, dtype=np.float32)
    out, _ = run_on_cores(x, w_qkv, w_proj, b_proj, mask)
    return out


# revision 22
# speedup vs baseline: 1.2570x; 1.1864x over previous
"""Sparse-attention (multiplicative mask) kernel for 8 TRN2 NeuronCores.

Reference computation (B=2, N=2048, C=1024, H=16, D=64):
    qkv = x @ w_qkv.T -> q,k,v [B,H,N,D]
    attn = softmax((q @ k.T) * D**-0.5 * mask)   # mask multiplies scores
    out  = (attn @ v) reshaped -> @ w_proj.T + b_proj

Sharding: core c handles batch b = c//4 and heads 4*(c%4) .. 4*(c%4)+3.
Each core computes a partial output projection (its 256 head-dims x full
w_proj columns); host sums the 4 partials per batch and adds b_proj.

Per-core device pipeline (bf16 matmul operands, fp32 softmax core):
  xT[c,n] resident; Q^T/K^T computed pair-packed [128,n] directly
  (M=128 matmuls spanning both heads' weight columns; scale folded
  into Q); V natural + ones column [128,nk,65] bf16.
  Attention per (pair, q-half): scores S^T[128,1024] on PE (row-tiled
  per head) -> DVE multiply by mask^T (bf16) -> ACT exp -> P^T bf16;
  AV accumulates out^T[65,1024] on PE two k-chunks behind the exp
  chain (deep software pipeline; the ones column of V gives the
  softmax denominator). At each q-half boundary av is quickly evicted
  to SBUF to free PSUM; the reciprocal + broadcast + normalize runs
  deferred, off the critical path. AO is pair-packed via SBUF->SBUF
  DMA for a K=128 output projection.
"""

import os

os.environ.setdefault("MYCRO_LOCAL_CACHE", "1")

import numpy as np
import ml_dtypes

import concourse.bass as bass
from concourse import bacc
import concourse.mybir as mybir
import concourse.tile as tile
from concourse.bass_utils import run_bass_kernel_spmd

B = 2
N = 2048
C = 1024
H = 16
D = 64
HPC = 4  # heads per core
NCORES = 8
SCALE = float(D) ** -0.5

F32 = mybir.dt.float32
BF16 = mybir.dt.bfloat16
EXP = mybir.ActivationFunctionType.Exp


def build_nc(n=N):
    """Build the per-core Bass program (SPMD; per-core inputs differ)."""
    nc = bacc.Bacc()

    NCC = C // 128        # contraction chunks for qkv proj (8)
    NK = n // 128         # key chunks (16)
    QH = n // 2           # half of the query range (1024)
    NQC = QH // 512       # 512-wide q chunks per half (2)

    xT = nc.dram_tensor("xT", [C, n], BF16, kind="ExternalInput")
    wqT = nc.dram_tensor("wqT", [C, HPC * D], BF16, kind="ExternalInput")
    wkT = nc.dram_tensor("wkT", [C, HPC * D], BF16, kind="ExternalInput")
    wvT = nc.dram_tensor("wvT", [C, HPC * D], BF16, kind="ExternalInput")
    wpT = nc.dram_tensor("wpT", [HPC * D, C], BF16, kind="ExternalInput")
    maskT = nc.dram_tensor("maskT", [n, n], BF16, kind="ExternalInput")
    y = nc.dram_tensor("y", [n, C], F32, kind="ExternalOutput")

    with tile.TileContext(nc) as tc:
        with (
            tc.tile_pool(name="weights", bufs=NCC) as wpool,
            tc.tile_pool(name="xt", bufs=NCC) as xpool,
            tc.tile_pool(name="qk", bufs=2) as qkpool,
            tc.tile_pool(name="vaug", bufs=HPC) as vpool,
            tc.tile_pool(name="ao", bufs=2) as aopool,
            tc.tile_pool(name="mask", bufs=4) as mpool,
            tc.tile_pool(name="work", bufs=4) as workpool,
            tc.tile_pool(name="small", bufs=4) as smallpool,
            tc.tile_pool(name="dscratch", bufs=4, space="DRAM") as dpool,
            tc.tile_pool(name="yout", bufs=3) as ypool,
        ):
            # ---- load weights + xT ----
            wq_sb = []
            wk_sb = []
            wv_sb = []
            x_sb = []
            for c in range(NCC):
                wq = wpool.tile([128, HPC * D], BF16, tag="wq")
                nc.sync.dma_start(out=wq, in_=wqT[c * 128:(c + 1) * 128, :])
                wq_sb.append(wq)
                wk = wpool.tile([128, HPC * D], BF16, tag="wk")
                nc.sync.dma_start(out=wk, in_=wkT[c * 128:(c + 1) * 128, :])
                wk_sb.append(wk)
                wv = wpool.tile([128, HPC * D], BF16, tag="wv")
                nc.sync.dma_start(out=wv, in_=wvT[c * 128:(c + 1) * 128, :])
                wv_sb.append(wv)
                xt = xpool.tile([128, n], BF16, tag="xt")
                nc.sync.dma_start(out=xt, in_=xT[c * 128:(c + 1) * 128, :])
                x_sb.append(xt)
            wp_sb = []
            for pair in range(HPC // 2):
                wp = wpool.tile([128, C], BF16, tag="wp", bufs=2)
                nc.sync.dma_start(
                    out=wp, in_=wpT[pair * 128:(pair + 1) * 128, :])
                wp_sb.append(wp)

            # ---- V for all heads (+ ones column), bf16 ----
            vaug = []
            with tc.tile_pool(name="v_psum", bufs=2, space="PSUM") as vps:
                for h in range(HPC):
                    va = vpool.tile([128, NK, D + 1], BF16, tag="vaug")
                    nc.vector.memset(va[:, :, D:D + 1], 1.0)
                    vaug.append(va)
                for quarter in range(NK // 4):
                    # each [128, 256] f32 output padded to its own 2KB PSUM
                    # bank: matmul start clears the whole bank region
                    vp = vps.tile([128, 4, 512], F32, tag="vps")
                    for kk in range(4):
                        k = quarter * 4 + kk
                        for c in range(NCC):
                            nc.tensor.matmul(
                                vp[:, kk, :HPC * D],
                                lhsT=x_sb[c][:, k * 128:(k + 1) * 128],
                                rhs=wv_sb[c],
                                start=(c == 0),
                                stop=(c == NCC - 1),
                            )
                    for h in range(HPC):
                        nc.vector.tensor_copy(
                            vaug[h][:, quarter * 4:(quarter + 1) * 4, :D],
                            vp[:, :, h * D:(h + 1) * D],
                        )

            # ---- Q^T / K^T pair-packed [128, n]: one M=128 matmul spans
            # both heads' weight columns, so head 2p lands on partitions
            # 0-63 and head 2p+1 on 64-127 directly ----
            qt2 = {}
            kt2 = {}
            with tc.tile_pool(name="qk_psum", bufs=4, space="PSUM") as qkps:
                for pair in range(HPC // 2):
                    hs = slice(pair * 128, (pair + 1) * 128)
                    for which, wsb, dstmap, scale in (
                        ("q", wq_sb, qt2, SCALE),
                        ("k", wk_sb, kt2, 1.0),
                    ):
                        dst = qkpool.tile([128, n], BF16, tag=f"{which}t",
                                          name=f"{which}t")
                        dstmap[pair] = dst
                        for qc in range(n // 512):
                            ps = qkps.tile([128, 512], F32, tag="qkps",
                                           name="qkps")
                            for c in range(NCC):
                                nc.tensor.matmul(
                                    ps,
                                    lhsT=wsb[c][:, hs],
                                    rhs=x_sb[c][:, qc * 512:(qc + 1) * 512],
                                    start=(c == 0),
                                    stop=(c == NCC - 1),
                                )
                            nc.vector.tensor_scalar_mul(
                                dst[:, qc * 512:(qc + 1) * 512], ps, scale)

            ao2 = {}
            for pair in range(HPC // 2):
                ao2[pair] = aopool.tile([128, n], BF16, tag="ao", name="ao2")

            # ---- attention ----
            deferred = []

            def emit_normalize(item):
                pair, qh, h, av_sb = item
                # reciprocal of the denominator row, reshaped to
                # [128, QH//128] via DRAM so all DVE lanes work
                rrd = dpool.tile([1, QH], F32, tag="rrd", name="rrd")
                nc.sync.dma_start(out=rrd, in_=av_sb[D:D + 1, :])
                grid = [[QH // 128, 128], [1, QH // 128]]
                rcol = smallpool.tile([128, QH // 128], F32,
                                      tag="rcol", name="rcol")
                nc.sync.dma_start(
                    out=rcol,
                    in_=bass.AP(tensor=rrd.tensor, offset=rrd.offset,
                                ap=[list(x) for x in grid]),
                )
                rco = smallpool.tile([128, QH // 128], F32,
                                     tag="rco", name="rco")
                nc.vector.reciprocal(rco, rcol)
                rd2 = dpool.tile([1, QH], F32, tag="rd2", name="rd2")
                nc.sync.dma_start(
                    out=bass.AP(tensor=rd2.tensor, offset=rd2.offset,
                                ap=[list(x) for x in grid]),
                    in_=rco,
                )
                bc = smallpool.tile([64, QH], F32, tag="bc", name="bc")
                nc.sync.dma_start(
                    out=bc,
                    in_=bass.AP(tensor=rd2.tensor, offset=rd2.offset,
                                ap=[[0, 64]] + rd2.ap[1:]),
                )
                if h % 2 == 0:
                    nc.vector.tensor_mul(
                        ao2[pair][0:64, qh * QH:(qh + 1) * QH],
                        av_sb[:D, :], bc)
                else:
                    tmp = smallpool.tile([64, QH], BF16, tag="aotmp",
                                         name="aotmp", bufs=2)
                    nc.vector.tensor_mul(tmp, av_sb[:D, :], bc)
                    # compute engines can't cross partitions; DMA packs the
                    # odd head onto partitions 64-127
                    nc.sync.dma_start(
                        out=ao2[pair][64:128, qh * QH:(qh + 1) * QH],
                        in_=tmp)

            with (
                tc.tile_pool(name="s_psum", bufs=2, space="PSUM") as sps,
                tc.tile_pool(name="av_psum", bufs=2, space="PSUM") as avps,
            ):
                for pair in range(HPC // 2):
                    heads = [2 * pair, 2 * pair + 1]
                    for qh in range(2):
                        av = {}
                        for h in heads:
                            av[h] = avps.tile([D + 1, QH], F32, tag="av",
                                              name="av")
                        # AV matmuls run two k behind the scores->mask->exp
                        # chain so PE never stalls on ACT
                        pending = []

                        def flush_av(batch):
                            for (fh, fk, fp) in batch:
                                for qc in range(NQC):
                                    nc.tensor.matmul(
                                        av[fh][:, qc * 512:(qc + 1) * 512],
                                        lhsT=vaug[fh][:, fk, :],
                                        rhs=fp[:, qc * 512:(qc + 1) * 512],
                                        start=(fk == 0),
                                        stop=(fk == NK - 1),
                                    )

                        for k in range(NK):
                            mt = mpool.tile([128, QH], BF16, tag="mask",
                                            name="mt")
                            nc.sync.dma_start(
                                out=mt,
                                in_=maskT[k * 128:(k + 1) * 128,
                                          qh * QH:(qh + 1) * QH],
                            )
                            s = {}
                            for i, h in enumerate(heads):
                                s[h] = sps.tile([128, QH], F32, tag="s",
                                                name="s")
                            for qc in range(NQC):
                                for i, h in enumerate(heads):
                                    nc.tensor.matmul(
                                        s[h][:, qc * 512:(qc + 1) * 512],
                                        lhsT=kt2[pair][i * 64:(i + 1) * 64,
                                                       k * 128:(k + 1) * 128],
                                        rhs=qt2[pair][i * 64:(i + 1) * 64,
                                                      qh * QH + qc * 512:
                                                      qh * QH + (qc + 1) * 512],
                                        start=True,
                                        stop=True,
                                        tile_position=(64 * i, 0),
                                    )
                            for h in heads:
                                t = workpool.tile([128, QH], F32, tag="t",
                                                  name="t")
                                nc.vector.tensor_mul(t, s[h], mt)
                                p = workpool.tile([128, QH], BF16, tag="p",
                                                  name="p", bufs=7)
                                nc.scalar.activation(p, t, EXP)
                                pending.append((h, k, p))
                            if len(pending) >= 6:
                                flush_av(pending[:2])
                                pending = pending[2:]
                        flush_av(pending)

                        # quick PSUM release: copy av to SBUF, defer the
                        # reciprocal/normalize chain off the critical path
                        for h in heads:
                            av_sb = smallpool.tile([D + 1, QH], F32,
                                                   tag="av_sb", name="av_sb",
                                                   bufs=4)
                            nc.scalar.copy(av_sb, av[h])
                            deferred.append((pair, qh, h, av_sb))
                        while len(deferred) > 2:
                            emit_normalize(deferred.pop(0))
            for item in deferred:
                emit_normalize(item)

            # ---- output projection (partial over this core's 256 dims) ----
            with tc.tile_pool(name="proj_psum", bufs=2, space="PSUM") as pps:
                for nt in range(n // 128):
                    pp = pps.tile([128, C], F32, tag="pp")
                    for pair in range(HPC // 2):
                        for ch in range(2):
                            nc.tensor.matmul(
                                pp[:, ch * 512:(ch + 1) * 512],
                                lhsT=ao2[pair][:, nt * 128:(nt + 1) * 128],
                                rhs=wp_sb[pair][:, ch * 512:(ch + 1) * 512],
                                start=(pair == 0),
                                stop=(pair == HPC // 2 - 1),
                            )
                    ysb = ypool.tile([128, C], F32, tag="ysb")
                    nc.scalar.copy(ysb, pp)
                    nc.sync.dma_start(out=y[nt * 128:(nt + 1) * 128, :], in_=ysb)

    nc.compile()
    return nc


def make_in_maps(x, w_qkv, mask, w_proj, n=N):
    """Host-side sharding: slice + transpose per-core inputs."""
    maskT = np.ascontiguousarray(mask.T).astype(ml_dtypes.bfloat16)
    xTs = [np.ascontiguousarray(x[b].T) for b in range(x.shape[0])]
    in_maps = []
    for core in range(NCORES):
        b = core // 4
        g = core % 4
        rows = slice(g * HPC * D, (g + 1) * HPC * D)
        bf = ml_dtypes.bfloat16
        in_maps.append({
            "xT": xTs[b].astype(bf),
            "wqT": np.ascontiguousarray(w_qkv[rows, :].T).astype(bf),
            "wkT": np.ascontiguousarray(w_qkv[C:2 * C][rows, :].T).astype(bf),
            "wvT": np.ascontiguousarray(w_qkv[2 * C:3 * C][rows, :].T).astype(bf),
            "wpT": np.ascontiguousarray(w_proj[:, rows].T).astype(bf),
            "maskT": maskT,
        })
    return in_maps


_NC_CACHE = {}


def _get_nc():
    if "nc" not in _NC_CACHE:
        _NC_CACHE["nc"] = build_nc()
    return _NC_CACHE["nc"]


def run_on_cores(x, w_qkv, w_proj, b_proj, mask, trace=False, trace_cores=None):
    nc = _get_nc()
    in_maps = make_in_maps(x, w_qkv, mask, w_proj)
    res = run_bass_kernel_spmd(
        nc,
        in_maps,
        core_ids=list(range(NCORES)),
        trace=trace,
        trace_cores=trace_cores,
    )
    out = np.zeros((B, N, C), dtype=np.float32)
    for core in range(NCORES):
        out[core // 4] += res.results[core]["y"]
    out += np.asarray(b_proj, dtype=np.float32)
    return out, res


def kernel(x, w_qkv, w_proj, b_proj, mask):
    x = np.asarray(x, dtype=np.float32)
    w_qkv = np.asarray(w_qkv, dtype=np.float32)
    w_proj = np.asarray(w_proj, dtype=np.float32)
    b_proj = np.asarray(b_proj, dtype=np.float32)
    mask = np.asarray(mask, dtype=np.float32)
    out, _ = run_on_cores(x, w_qkv, w_proj, b_proj, mask)
    return out
